# revision 2
# baseline (speedup 1.0000x reference)
"""Trainium2 Bass kernel for nn_EnhanceSelfAttention (B=16, N=577, C=768, H=12).

Self-contained: takes full unsharded inputs, shards batch across 8 NeuronCores
(2 batches/core), runs a fused attention kernel per core, gathers the output.

v2: host-side data staging removes all on-device layout work:
  - x is transposed + cast to f16 on host -> xT [128, 6*1154] (no PE transposes)
  - qkv/out weights repacked per 128-row chunk so every load is one
    contiguous-run DMA
  - the relative-position bias + causal mask are exp()'d on host and shipped
    as per-k-tile f16 tables in the exact SBUF layout (no gather + ScalarE exp)
  - softmax scale folded into the q weights/bias on host

Per-core pipeline (f16 matmul operands, fp32 PSUM):
  B. qT,kT = Wqk^T @ xT per head-pair, interleaved into D as PE gap filler
  C. v = x @ Wv + b stored per k-tile in [k, 12*65] f16 (ones col -> denom)
  D. attention per head-pair: sT = kT.T@qT (two heads via PE row groups),
     p = exp(sT) * expb, OT += v.T@p over causal k-tiles; row 64 of OT is the
     denominator; divide via DVE reciprocal + GpSimd partition-broadcast.
  E. y = OT.T @ Wout + b, two merged strided DMAs per batch to DRAM.
"""

import numpy as np
import ml_dtypes

import concourse.bass as bass
import concourse.tile as tile
from concourse import bacc, mybir
from concourse.bass_utils import run_bass_kernel_spmd

F32 = mybir.dt.float32
F16 = mybir.dt.float16

B, NTOK, CDIM, NH, DH = 16, 577, 768, 12, 64
GRID = 24
NRD = (2 * GRID - 1) * (2 * GRID - 1) + 3  # 2212
NCORES = 8
BLOC = B // NCORES       # batches per core
NSEQ = BLOC * NTOK       # 1154
SCALE = DH ** -0.5       # 0.125
NEG = -65504.0

QBLOCKS = [(0, 128), (128, 449)]            # (qstart, qN)
KTILES = [(0, 128), (128, 128), (256, 128), (384, 128), (512, 65)]
QLO = [k0 for k0, _ in KTILES]              # per-tile stored q range [QLO[t], 577)
WID = [NTOK - q for q in QLO]               # 577, 449, 321, 193, 65
NBLK = [(0, 386), (386, 384), (770, 384)]   # token blocks for B projections

EXB_OFF = []
_off = 0
for _t, (_k0, _pw) in enumerate(KTILES):
    EXB_OFF.append(_off)
    _off += _pw * WID[_t] * NH
EXB_TOTAL = _off

_CACHE = {}


def _check_rel_index(ri):
    """Assert the Toeplitz structure of rel_index (sanity only)."""
    assert ri.shape == (NTOK, NTOK)
    assert ri[0, 0] == NRD - 1
    assert (ri[0, 1:] == NRD - 3).all()
    assert (ri[1:, 0] == NRD - 2).all()


def _host_prep(x, qkv_w, qkv_b, pos_emb, out_w, out_b, ri):
    """Build all per-core DMA images on host."""
    _check_rel_index(ri)
    f16 = np.float16

    # exp(bias + causal mask) tables, one per k-tile, in SBUF layout
    # expb[t][p, h*W + (q - qlo)] = exp(pos_emb[h, ri[q, k0+p]] + mask(q, k0+p))
    bias = pos_emb[:, ri]                                   # [12, 577, 577]
    mask = np.where(np.tri(NTOK, dtype=np.float32) > 0, 0.0, NEG)
    exb_full = np.exp(bias + mask[None]).astype(f16)        # [12, q, k]
    exb = np.empty(EXB_TOTAL, dtype=f16)
    for t, (k0, pw) in enumerate(KTILES):
        qlo = QLO[t]
        blk = exb_full[:, qlo:, k0:k0 + pw]                 # [12, W, pw]
        exb[EXB_OFF[t]:EXB_OFF[t] + pw * WID[t] * NH] = (
            np.ascontiguousarray(blk.transpose(2, 0, 1)).reshape(-1))

    # weights: per-chunk contiguous images
    wq = (qkv_w[:, :CDIM] * SCALE).astype(f16)              # scale folded in
    wk = qkv_w[:, CDIM:2 * CDIM].astype(f16)
    wv = qkv_w[:, 2 * CDIM:].astype(f16)
    wo = out_w.astype(f16)

    def chunked(w):  # [768, 768] -> [128, 6*768]: out[p, c*768+d] = w[c*128+p, d]
        return np.ascontiguousarray(
            w.reshape(6, 128, CDIM).transpose(1, 0, 2)).reshape(128, 6 * CDIM)

    # hqk[jp] = [128, 1536]: cols 0:768 q-chunk jp, cols 768:1536 k-chunk jp
    # hqk[jp][p, part*768 + c*128 + d] = w[c*128+p, jp*128+d]
    hqk = np.empty((6, 128, 2 * CDIM), dtype=f16)
    for jp in range(6):
        for part, w in ((0, wq), (1, wk)):
            blk = w[:, jp * 128:(jp + 1) * 128]             # [768, 128]
            hqk[jp, :, part * CDIM:(part + 1) * CDIM] = (
                blk.reshape(6, 128, 128).transpose(1, 0, 2).reshape(128, CDIM))
    hqk = np.ascontiguousarray(hqk.reshape(6 * 128, 2 * CDIM))
    hv = chunked(wv)
    hwo = chunked(wo)

    # bias columns for q/k activations: [128, 12] f32
    hbc = np.empty((128, 12), dtype=np.float32)
    for r in range(12):
        if r < 6:
            hbc[:, r] = qkv_b[r * 128:(r + 1) * 128] * SCALE
        else:
            hbc[:, r] = qkv_b[CDIM + (r - 6) * 128:CDIM + (r - 5) * 128]
    # bias rows: [1, 1536] f16 = [v-bias | out-bias]
    hbrow = np.concatenate([qkv_b[2 * CDIM:], out_b]).astype(f16)[None, :]

    # per-core x images: [128, 6*1154], hx[p, c*1154+t] = xshard[t, c*128+p]
    hx = []
    for c in range(NCORES):
        shard = x[c * BLOC:(c + 1) * BLOC].reshape(NSEQ, CDIM)
        hx.append(np.ascontiguousarray(
            shard.T.astype(f16).reshape(6, 128, NSEQ).transpose(1, 0, 2)
        ).reshape(128, 6 * NSEQ))
    return exb, hqk, hv, hwo, hbc, hbrow, hx


def _build():
    nc = bacc.Bacc("TRN2", target_bir_lowering=False, debug=False)

    hx_d = nc.dram_tensor("hx", [128, 6 * NSEQ], F16, kind="ExternalInput").ap()
    hqk_d = nc.dram_tensor("hqk", [6 * 128, 2 * CDIM], F16,
                           kind="ExternalInput").ap()
    hv_d = nc.dram_tensor("hv", [128, 6 * CDIM], F16, kind="ExternalInput").ap()
    hwo_d = nc.dram_tensor("hwo", [128, 6 * CDIM], F16, kind="ExternalInput").ap()
    hbc_d = nc.dram_tensor("hbc", [128, 12], F32, kind="ExternalInput").ap()
    hbrow_d = nc.dram_tensor("hbrow", [1, 2 * CDIM], F16,
                             kind="ExternalInput").ap()
    exb_d = nc.dram_tensor("exb", [EXB_TOTAL], F16, kind="ExternalInput").ap()
    y_d = nc.dram_tensor("y", [NSEQ, CDIM], F32, kind="ExternalOutput").ap()

    with tile.TileContext(nc) as tc:
        _emit(nc, tc, hx_d, hqk_d, hv_d, hwo_d, hbc_d, hbrow_d, exb_d, y_d)
    nc.compile()
    return nc


def _emit(nc, tc, hx_d, hqk_d, hv_d, hwo_d, hbc_d, hbrow_d, exb_d, y_d):
    from contextlib import ExitStack

    with ExitStack() as top:
        persist = top.enter_context(tc.tile_pool(name="persist", bufs=1))
        consts = top.enter_context(tc.tile_pool(name="consts", bufs=1))

        # ---- persistent tiles ----
        xT = consts.tile([128, 6 * NSEQ], F16, tag="xT", name="xT")
        wqk = [consts.tile([128, 2 * CDIM], F16, tag=f"wqk{j}", name=f"wqk{j}")
               for j in range(6)]
        qkb = consts.tile([128, 12], F32, tag="qkb", name="qkb")
        brow = consts.tile([1, 2 * CDIM], F16, tag="brow", name="brow")
        ones128 = consts.tile([1, 128], F16, tag="ones128", name="ones128")

        qT = [persist.tile([128, NSEQ], F16, tag=f"qT{j}", name=f"qT{j}")
              for j in range(6)]
        kT = [persist.tile([128, NSEQ], F16, tag=f"kT{j}", name=f"kT{j}")
              for j in range(6)]
        vt = [[persist.tile([128, NH * 65], F16, tag=f"v{b}_{t}", name=f"v{b}_{t}")
               for t in range(5)] for b in range(BLOC)]
        oT = [persist.tile([128, NSEQ], F16, tag=f"oT{j}", name=f"oT{j}")
              for j in range(6)]
        expb = [persist.tile([128, WID[t] * NH], F16, tag=f"expb{t}",
                             name=f"expb{t}") for t in range(5)]
        vbias = consts.tile([128, CDIM], F32, tag="vbias", name="vbias")
        obias = consts.tile([128, CDIM], F32, tag="obias", name="obias")

        # ---- DMA issue: sync(SP) queue, ordered for earliest B start ----
        nc.sync.dma_start(qkb[:], hbc_d[:, :])
        nc.sync.dma_start(brow[:], hbrow_d[:, :])
        xT3 = xT[:].rearrange("p (c t) -> p c t", c=6)
        nc.sync.dma_start(wqk[0][:], hqk_d[0:128, :])
        nc.sync.dma_start(
            xT3[:, :, NBLK[0][0]:NBLK[0][0] + NBLK[0][1]],
            bass.AP(hx_d.tensor, NBLK[0][0],
                    [[6 * NSEQ, 128], [NSEQ, 6], [1, NBLK[0][1]]]))
        nc.sync.dma_start(wqk[1][:], hqk_d[128:256, :])
        wvall = consts.tile([128, 6 * CDIM], F16, tag="wvall", name="wvall")
        nc.sync.dma_start(wvall[:], hv_d[:, :])
        for nb in (1, 2):
            nb0, nbw = NBLK[nb]
            nc.sync.dma_start(
                xT3[:, :, nb0:nb0 + nbw],
                bass.AP(hx_d.tensor, nb0, [[6 * NSEQ, 128], [NSEQ, 6], [1, nbw]]))
        for j in range(2, 6):
            nc.sync.dma_start(wqk[j][:], hqk_d[j * 128:(j + 1) * 128, :])
        woall = consts.tile([128, 6 * CDIM], F16, tag="woall", name="woall")
        nc.sync.dma_start(woall[:], hwo_d[:, :])

        # exp-bias tables on the gpsimd (Pool) queue, in first-use order
        for t, (k0, pw) in enumerate(KTILES):
            src = bass.AP(exb_d.tensor, EXB_OFF[t],
                          [[WID[t] * NH, pw], [1, WID[t] * NH]])
            nc.gpsimd.dma_start(expb[t][0:pw, :], src)

        nc.vector.memset(ones128[:], 1.0)

        # broadcast bias rows -> [128, 768] tiles (PE, cheap)
        with tc.tile_pool(name="bb_psum", bufs=2, space="PSUM") as bbps:
            for bi, dst in ((0, vbias), (1, obias)):
                for h0 in (0, 384):
                    ps = bbps.tile([128, 384], F32, tag="bb", name="bb")
                    nc.tensor.matmul(ps[:], ones128[:],
                                     brow[0:1, bi * CDIM + h0:bi * CDIM + h0 + 384],
                                     start=True, stop=True)
                    nc.vector.tensor_copy(dst[:, h0:h0 + 384], ps[:])

        # ---------------- emission helpers ----------------
        def b_units(jp, ps_qk):
            """Yield 6 emission units for head-pair jp's q/k projections."""
            for r in (jp, jp + 6):
                part = 0 if r < 6 else 1
                dst = qT[jp] if r < 6 else kT[jp]
                for nb0, nbw in NBLK:
                    def unit(r=r, part=part, dst=dst, nb0=nb0, nbw=nbw):
                        ps = ps_qk.tile([128, 386], F32, tag="psqk", name="psqk")
                        for c in range(6):
                            nc.tensor.matmul(
                                ps[0:128, 0:nbw],
                                wqk[jp if r < 6 else r - 6][
                                    :, part * CDIM + c * 128:
                                    part * CDIM + (c + 1) * 128],
                                xT3[:, c, nb0:nb0 + nbw],
                                start=(c == 0), stop=(c == 5))
                        nc.scalar.activation(
                            dst[:, nb0:nb0 + nbw], ps[0:128, 0:nbw],
                            mybir.ActivationFunctionType.Identity,
                            bias=qkb[:, r:r + 1])
                    yield unit

        with tc.tile_pool(name="ps_qk", bufs=2, space="PSUM") as ps_qk:
            # ---- B(0) ----
            for u in b_units(0, ps_qk):
                u()

            # ---- C: v projection ----
            with tc.tile_pool(name="ps_v", bufs=2, space="PSUM") as ps_v:
                for b in range(BLOC):
                    for t, (k0, pw) in enumerate(KTILES):
                        vtile = vt[b][t]
                        for half in range(2):
                            ps = ps_v.tile([128, 384], F32, tag="psv", name="psv")
                            for c in range(6):
                                nc.tensor.matmul(
                                    ps[0:pw, :],
                                    xT3[:, c, b * NTOK + k0:b * NTOK + k0 + pw],
                                    wvall[:, c * CDIM + half * 384:
                                          c * CDIM + (half + 1) * 384],
                                    start=(c == 0), stop=(c == 5))
                            dst = vtile[0:pw, :].rearrange(
                                "p (h d) -> p h d",
                                h=NH)[:, half * 6:(half + 1) * 6, 0:64]
                            src = ps[0:pw, :].rearrange("p (h d) -> p h d", d=64)
                            bsl = vbias[0:pw, half * 384:(half + 1) * 384].rearrange(
                                "p (h d) -> p h d", d=64)
                            nc.vector.tensor_tensor(out=dst, in0=src, in1=bsl,
                                                    op=mybir.AluOpType.add)
                        nc.vector.memset(
                            vtile[0:pw, :].rearrange("p (h d) -> p h d",
                                                     h=NH)[:, :, 64:65], 1.0)

            # ---- D per head-pair, with B(jp+1) interleaved ----
            with tc.tile_pool(name="ps_sT", bufs=4, space="PSUM") as ps_sT, \
                 tc.tile_pool(name="ps_OT", bufs=2, space="PSUM") as ps_OT, \
                 tc.tile_pool(name="att_tmp", bufs=8) as att_tmp, \
                 tc.tile_pool(name="es_pool", bufs=8) as es_pool, \
                 tc.tile_pool(name="p_pool", bufs=8) as p_pool:
                for jp in range(6):
                    units = list(b_units(jp + 1, ps_qk)) if jp < 5 else []

                    def fill(n):
                        for _ in range(n):
                            if units:
                                units.pop(0)()

                    # qb0 for both batches shares one psO tile (cols 0:128 b0,
                    # 128:256 b1); qb1 gets one tile per batch.
                    psO_qb0 = [ps_OT.tile([65, 456], F32, tag="psOT",
                                          name="psOT") for _ in range(2)]

                    def attn(b, qstart, qN, psO, ocol):
                        """Emit S/exp/mult/AV pipeline for one (b, q-block)."""
                        qend = qstart + qN
                        valid_t = [t for t in range(5) if QLO[t] < qend]
                        tlast = valid_t[-1]
                        prev = None

                        def av(t):
                            k0, pw = KTILES[t]
                            qlo = max(qstart, QLO[t])
                            off = ocol + qlo - qstart
                            Nt = qend - qlo
                            for side in range(2):
                                h = 2 * jp + side
                                nc.tensor.matmul(
                                    psO[side][0:65, off:off + Nt],
                                    vt[b][t][0:pw, h * 65:(h + 1) * 65],
                                    pt[t][side][0:pw, 0:Nt],
                                    start=(t == valid_t[0]),
                                    stop=(t == tlast))

                        pt = {}
                        for i, t in enumerate(valid_t):
                            k0, pw = KTILES[t]
                            qlo = max(qstart, QLO[t])
                            Nt = qend - qlo
                            ebase = qlo - QLO[t]
                            psS = [ps_sT.tile([128, 456], F32, tag="psS",
                                              name="psS") for _ in range(2)]
                            for side in range(2):
                                r0 = side * 64
                                nc.tensor.matmul(
                                    psS[side][0:pw, 0:Nt],
                                    kT[jp][r0:r0 + 64,
                                           b * NTOK + k0:b * NTOK + k0 + pw],
                                    qT[jp][r0:r0 + 64,
                                           b * NTOK + qlo:b * NTOK + qlo + Nt],
                                    start=True, stop=True,
                                    tile_position=(r0, 0))
                            pt[t] = []
                            for side in range(2):
                                h = 2 * jp + side
                                es = es_pool.tile([128, 456], F16, tag="es",
                                                  name="es")
                                nc.scalar.activation(
                                    es[0:pw, 0:Nt], psS[side][0:pw, 0:Nt],
                                    mybir.ActivationFunctionType.Exp)
                                p = p_pool.tile([128, 456], F16, tag="p",
                                                name="p")
                                nc.vector.tensor_tensor(
                                    out=p[0:pw, 0:Nt],
                                    in0=es[0:pw, 0:Nt],
                                    in1=expb[t][0:pw,
                                                h * WID[t] + ebase:
                                                h * WID[t] + ebase + Nt],
                                    op=mybir.AluOpType.mult)
                                pt[t].append(p)
                            if prev is not None:
                                av(prev)
                                del pt[prev]
                                if i % 2 == 0:
                                    fill(1)
                            prev = t
                        av(prev)

                        def norm():
                            for side in range(2):
                                recip = att_tmp.tile([1, 456], F32, tag="recip",
                                                     name="recip")
                                nc.vector.reciprocal(
                                    recip[0:1, 0:qN],
                                    psO[side][64:65, ocol:ocol + qN])
                                rb = att_tmp.tile([64, 456], F32, tag="rb",
                                                  name="rb")
                                nc.gpsimd.partition_broadcast(rb[0:64, 0:qN],
                                                              recip[0:1, 0:qN])
                                r0 = side * 64
                                nc.vector.tensor_tensor(
                                    out=oT[jp][r0:r0 + 64,
                                               b * NTOK + qstart:b * NTOK + qend],
                                    in0=psO[side][0:64, ocol:ocol + qN],
                                    in1=rb[0:64, 0:qN],
                                    op=mybir.AluOpType.mult)
                        return norm

                    # qb0 (tiny) for both batches first: overlapping chains
                    n0 = attn(0, QBLOCKS[0][0], QBLOCKS[0][1], psO_qb0, 0)
                    fill(1)
                    n1 = attn(1, QBLOCKS[0][0], QBLOCKS[0][1], psO_qb0, 128)
                    n0()
                    fill(1)
                    n1()
                    # qb1 per batch
                    for b in range(BLOC):
                        psO = [ps_OT.tile([65, 456], F32, tag="psOT",
                                          name="psOT") for _ in range(2)]
                        nrm = attn(b, QBLOCKS[1][0], QBLOCKS[1][1], psO, 0)
                        fill(2)
                        nrm()
                    fill(6)

        # ---------------- E: output projection ----------------
        with tc.tile_pool(name="ps_o", bufs=3, space="PSUM") as ps_o, \
             tc.tile_pool(name="out_sb", bufs=1) as out_sb:
            for b in range(BLOC):
                ot = out_sb.tile([128, 5 * CDIM], F32, tag=f"ot{b}",
                                 name=f"ot{b}")
                for mi, m0 in enumerate(range(0, NTOK, 128)):
                    mw = min(128, NTOK - m0)
                    for half in range(2):
                        ps = ps_o.tile([128, 384], F32, tag="pso", name="pso")
                        for c in range(6):
                            nc.tensor.matmul(
                                ps[0:mw, :],
                                oT[c][:, b * NTOK + m0:b * NTOK + m0 + mw],
                                woall[:, c * CDIM + half * 384:
                                      c * CDIM + (half + 1) * 384],
                                start=(c == 0), stop=(c == 5))
                        nc.vector.tensor_tensor(
                            out=ot[0:mw, mi * CDIM + half * 384:
                                   mi * CDIM + (half + 1) * 384],
                            in0=ps[0:mw, :],
                            in1=obias[0:mw, half * 384:(half + 1) * 384],
                            op=mybir.AluOpType.add)
                # merged stores: m-tiles 0-3 in one DMA, ragged tail separate
                nc.sync.dma_start(
                    bass.AP(y_d.tensor, b * NTOK * CDIM,
                            [[CDIM, 128], [128 * CDIM, 4], [1, CDIM]]),
                    ot[0:128, 0:4 * CDIM].rearrange("p (m f) -> p m f", m=4))
                nc.sync.dma_start(
                    bass.AP(y_d.tensor, (b * NTOK + 512) * CDIM,
                            [[CDIM, 65], [1, CDIM]]),
                    ot[0:65, 4 * CDIM:5 * CDIM])


def kernel(x, qkv_w, qkv_b, pos_emb, out_w, out_b, rel_index):
    x = np.asarray(x, dtype=np.float32)
    qkv_w = np.asarray(qkv_w, dtype=np.float32)
    qkv_b = np.asarray(qkv_b, dtype=np.float32)
    pos_emb = np.asarray(pos_emb, dtype=np.float32)
    out_w = np.asarray(out_w, dtype=np.float32)
    out_b = np.asarray(out_b, dtype=np.float32)
    ri = np.asarray(rel_index, dtype=np.int32)

    if "nc" not in _CACHE:
        _CACHE["nc"] = _build()
    nc = _CACHE["nc"]

    exb, hqk, hv, hwo, hbc, hbrow, hx = _host_prep(
        x, qkv_w, qkv_b, pos_emb, out_w, out_b, ri)
    in_maps = []
    for c in range(NCORES):
        in_maps.append({
            "hx": hx[c], "hqk": hqk, "hv": hv, "hwo": hwo,
            "hbc": hbc, "hbrow": hbrow, "exb": exb,
        })
    res = run_bass_kernel_spmd(nc, in_maps, core_ids=list(range(NCORES)))
    out = np.empty((B, NTOK, CDIM), dtype=np.float32)
    for c in range(NCORES):
        out[c * BLOC:(c + 1) * BLOC] = res.results[c]["y"].reshape(BLOC, NTOK, CDIM)
    return out
